# revision 11
# baseline (speedup 1.0000x reference)
"""GAT (2-layer, 2-head, global-softmax) Trainium2 kernel over 8 NeuronCores.

Strategy: nodes partitioned by destination across 8 cores (6250 dst each);
edges live with their dst owner. The GLOBAL softmax factorizes: out =
[sum_e exp(s_e) h_src] / gsum with gsum a per-head global scalar, so each
layer is ONE pass over edges plus a tiny AllReduce (max-subtraction dropped;
it only guards a 1e-10 epsilon that is ~1e-15 relative here).

Layout/engine assignment (v2, rebuilt from NTFF profile of v1):
  - table [N, 256] bf16 rows (512B, gather granularity), only cols 0:130
    written (h 0:128, asrc 128:130) -> table-write traffic halved.
  - adst table bf16 [NLOC+1, 128] (256B rows, gather min) with -1e9 sentinel.
  - build phases: host passes x^T in bf16; lhsT slabs of 8 tiles per DMA
    (sync-engine issue count was the v1 build bottleneck); stores issued on
    the Activation engine's HWDGE queue; PSUM->SBUF copies fused (h|asrc in
    one copy).
  - edge pass per supertile: 3 dma_gather calls (994ns fixed cost each was
    dominant at v1's 7 calls), one batched is_equal builds all nb one-hot
    blocks (v1: 20 separate ops), lrelu/exp/U-copy on the Scalar engine,
    message multiply as one broadcast op. Segment-sum via one-hot matmul
    accumulation in PSUM (unchanged).
  - layer-1 activations are transposed at fixup (50 PE transposes) and
    AllGathered as [C, NLOC] so layer-2 table build needs no transposes.
"""
import sys

sys.path.insert(0, "/opt/trn_rl_repo")

import numpy as np

N = 50000
FIN = 128
C = 64
H = 2
HC = H * C  # 128
E = 800000
N_CORES = 8
NLOC = N // N_CORES          # 6250
NST_NODES = 125              # dst nodes per supertile
S = NLOC // NST_NODES        # 50 supertiles per core
S_GLOBAL = S * N_CORES       # 400
V_HALF = N // 2              # 25000 rows per table half (int16-safe indices)
TROW = 256                   # bf16 elems per table row (512 B): h[0:128], asrc[128:130]
AROW = 128                   # bf16 elems per adst row (256 B): adst[0:2]
SENT_A = NLOC                # adst sentinel row (-1e9)

_compiled = None  # (nc, (nb_lo, nb_hi))


# --------------------------------------------------------------------------
# host-side graph preprocessing (pure index manipulation)
# --------------------------------------------------------------------------

def _wrap_idx(flat):
    """[n] -> [128, n/16] int16 wrapped + 8x replicated layout for dma_gather."""
    w = np.asarray(flat, np.int16).reshape(-1, 16).T
    return np.tile(w, (8, 1))


def _preprocess(edge_index):
    src = np.concatenate([edge_index[0].astype(np.int64), np.arange(N, dtype=np.int64)])
    dst = np.concatenate([edge_index[1].astype(np.int64), np.arange(N, dtype=np.int64)])
    order = np.argsort(dst, kind="stable")
    src, dst = src[order], dst[order]

    stg = (dst // NST_NODES).astype(np.int64)          # global supertile id, sorted
    starts = np.searchsorted(stg, np.arange(S_GLOBAL))
    ends = np.searchsorted(stg, np.arange(S_GLOBAL), side="right")
    lo_mask = src < V_HALF
    n_lo = np.array([int(lo_mask[a:b].sum()) for a, b in zip(starts, ends)])
    n_hi = (ends - starts) - n_lo
    nb_lo = int(np.ceil(n_lo.max() / 128))
    nb_hi = int(np.ceil(n_hi.max() / 128))
    cap_lo, cap_hi = nb_lo * 128, nb_hi * 128
    nb = nb_lo + nb_hi
    cap = cap_lo + cap_hi

    ilo = np.zeros((N_CORES, S, 128, cap_lo // 16), np.int16)
    ihi = np.zeros((N_CORES, S, 128, cap_hi // 16), np.int16)
    ia = np.zeros((N_CORES, S, 128, cap // 16), np.int16)
    slot = np.zeros((N_CORES, S, 128, nb), np.float32)

    for g in range(S_GLOBAL):
        k, t = divmod(g, S)
        a, b = starts[g], ends[g]
        s_src, s_dst = src[a:b], dst[a:b]
        m = s_src < V_HALF
        src_lo, dst_lo = s_src[m], s_dst[m]
        src_hi, dst_hi = s_src[~m], s_dst[~m]

        i_lo = np.zeros(cap_lo, np.int64)
        i_lo[: len(src_lo)] = src_lo
        i_hi = np.zeros(cap_hi, np.int64)
        i_hi[: len(src_hi)] = src_hi - V_HALF

        d_all = np.full(cap, SENT_A, np.int64)
        d_all[: len(dst_lo)] = dst_lo - NLOC * k
        d_all[cap_lo : cap_lo + len(dst_hi)] = dst_hi - NLOC * k

        sl = np.full(cap, 127, np.int64)
        sl[: len(dst_lo)] = dst_lo - NST_NODES * g
        sl[cap_lo : cap_lo + len(dst_hi)] = dst_hi - NST_NODES * g

        ilo[k, t] = _wrap_idx(i_lo)
        ihi[k, t] = _wrap_idx(i_hi)
        ia[k, t] = _wrap_idx(d_all)
        slot[k, t] = sl.reshape(nb, 128).T.astype(np.float32)

    return ilo, ihi, ia, slot, nb_lo, nb_hi


# --------------------------------------------------------------------------
# device program
# --------------------------------------------------------------------------

def _build_program(nb_lo, nb_hi, phases=7):
    import concourse.bass as bass
    import concourse.bacc as bacc
    import concourse.mybir as mybir
    from concourse import library_config
    from concourse.masks import make_identity
    from concourse.tile import TileContext

    f32 = mybir.dt.float32
    bf16 = mybir.dt.bfloat16
    i16 = mybir.dt.int16
    i32 = mybir.dt.int32
    Alu = mybir.AluOpType
    Act = mybir.ActivationFunctionType

    nb = nb_lo + nb_hi
    cap_lo, cap_hi, cap = nb_lo * 128, nb_hi * 128, (nb_lo + nb_hi) * 128

    nc = bacc.Bacc("TRN2", target_bir_lowering=False, debug=False,
                   num_devices=N_CORES, num_swdge_queues=4)

    # ---- I/O
    xT_in = nc.declare_dram_parameter("xT", [FIN, N], bf16, isOutput=False)
    xTo_in = nc.declare_dram_parameter("xTo", [FIN, NLOC], bf16, isOutput=False)
    w1_in = nc.declare_dram_parameter("W1", [HC, FIN], f32, isOutput=False)
    w2_in = nc.declare_dram_parameter("W2", [HC, C], f32, isOutput=False)
    att1_in = nc.declare_dram_parameter("att1", [1, H, 2 * C], f32, isOutput=False)
    att2_in = nc.declare_dram_parameter("att2", [1, H, 2 * C], f32, isOutput=False)
    b1_in = nc.declare_dram_parameter("b1", [C], f32, isOutput=False)
    b2_in = nc.declare_dram_parameter("b2", [C], f32, isOutput=False)
    ilo_in = nc.declare_dram_parameter("ilo", [S, 128, cap_lo // 16], i16, isOutput=False)
    ihi_in = nc.declare_dram_parameter("ihi", [S, 128, cap_hi // 16], i16, isOutput=False)
    ia_in = nc.declare_dram_parameter("ia", [S, 128, cap // 16], i16, isOutput=False)
    slot_in = nc.declare_dram_parameter("slot", [S, 128, nb], f32, isOutput=False)
    out_ext = nc.declare_dram_parameter("out", [NLOC, C], f32, isOutput=True)

    # ---- internal DRAM
    table = nc.dram_tensor("table_d", [N, TROW], bf16)
    adst_tbl = nc.dram_tensor("adst_d", [NLOC + 1, AROW], bf16)
    ar_in = nc.dram_tensor("ar_in_d", [1, H], f32)
    ar_out = nc.dram_tensor("ar_out_d", [1, H], f32, addr_space="Shared")
    ag_in = nc.dram_tensor("ag_in_d", [C, NLOC], bf16)
    actT_full = nc.dram_tensor("actT_full_d", [N_CORES * C, NLOC], bf16, addr_space="Shared")

    G = 8  # tiles per build slab

    with TileContext(nc) as tc:
        with (
            tc.tile_pool(name="const", bufs=1) as cpool,
            tc.tile_pool(name="bld", bufs=3) as bld,
            tc.tile_pool(name="bldp", bufs=2, space="PSUM") as bldp,
            tc.tile_pool(name="gat", bufs=2) as gat,
            tc.tile_pool(name="edge", bufs=2) as edge,
            tc.tile_pool(name="up", bufs=2, space="PSUM") as upool,
            tc.tile_pool(name="fix", bufs=1) as fix,
        ):
            nc.gpsimd.load_library(library_config.mlp)

            # ============ constants ============
            ident = cpool.tile([128, 128], bf16)
            make_identity(nc, ident[:])
            iota_i = cpool.tile([128, 128], i32)
            nc.gpsimd.iota(iota_i[:], pattern=[[1, 128]], base=0, channel_multiplier=0)
            iota3 = cpool.tile([128, 1, 128], bf16)
            nc.vector.tensor_copy(out=iota3[:, 0, :], in_=iota_i[:])
            ones_row = cpool.tile([1, 128], f32)
            nc.vector.memset(ones_row[:], 1.0)
            ones_col = cpool.tile([128, 1], f32)
            nc.vector.memset(ones_col[:], 1.0)

            # all idx/slot arrays, resident (layer-independent)
            ilo_all = cpool.tile([128, S, cap_lo // 16], i16)
            ihi_all = cpool.tile([128, S, cap_hi // 16], i16)
            ia_all = cpool.tile([128, S, cap // 16], i16)
            nc.sync.dma_start(out=ilo_all[:], in_=ilo_in.ap().rearrange("s p w -> p s w"))
            nc.sync.dma_start(out=ihi_all[:], in_=ihi_in.ap().rearrange("s p w -> p s w"))
            nc.sync.dma_start(out=ia_all[:], in_=ia_in.ap().rearrange("s p w -> p s w"))
            slot4 = cpool.tile([128, S, nb, 1], f32)
            nc.sync.dma_start(
                out=slot4[:, :, :, 0], in_=slot_in.ap().rearrange("s p w -> p s w")
            )

            # own-node x^T, resident (adst layer-1 lhsT)
            xto = cpool.tile([128, NLOC], bf16)
            nc.sync.dma_start(out=xto[:], in_=xTo_in[:, :])

            # bias broadcast tiles [128, 1, C]
            b1b = cpool.tile([128, 1, C], f32)
            nc.sync.dma_start(out=b1b[:, 0, :], in_=b1_in.ap().partition_broadcast(128))
            b2b = cpool.tile([128, 1, C], f32)
            nc.sync.dma_start(out=b2b[:, 0, :], in_=b2_in.ap().partition_broadcast(128))

            # sentinel row of adst table
            sent_t = cpool.tile([1, AROW], bf16)
            nc.vector.memset(sent_t[:], -1.0e9)
            nc.sync.dma_start(out=adst_tbl[SENT_A : SENT_A + 1, :], in_=sent_t[:])

            # ---- R matrices: R = [W^T | Vsrc' | Vdst']  (bf16, [K=kdim, 132])
            def build_R(w_dram, att_dram, kdim):
                wt = bld.tile([128, kdim], f32, tag="wld")
                nc.sync.dma_start(out=wt[:, :], in_=w_dram[:])          # [HC, kdim]
                wb = bld.tile([128, kdim], bf16, tag="wldb")
                nc.vector.tensor_copy(out=wb[:], in_=wt[:])
                vsd = cpool.tile([128, 4], f32, tag="vsd")  # cols 0:2 Vsrc, 2:4 Vdst
                nc.vector.memset(vsd[:], 0.0)
                for h in range(H):
                    nc.sync.dma_start(
                        out=vsd[h * C : (h + 1) * C, h : h + 1],
                        in_=att_dram[0:1, h, C : 2 * C].rearrange("o c -> c o"),
                    )
                    nc.sync.dma_start(
                        out=vsd[h * C : (h + 1) * C, 2 + h : 3 + h],
                        in_=att_dram[0:1, h, 0:C].rearrange("o c -> c o"),
                    )
                vsdb = cpool.tile([128, 4], bf16, tag="vsdb")
                nc.vector.tensor_copy(out=vsdb[:], in_=vsd[:])
                r_ps = bldp.tile([128, 132], f32, tag="ps", space="PSUM")
                nc.tensor.transpose(
                    out=r_ps[:kdim, 0:128].bitcast(bf16)[:, 0:128], in_=wb[:, :],
                    identity=ident[:],
                )
                nc.tensor.matmul(out=r_ps[:kdim, 128:132], lhsT=wb[:, :], rhs=vsdb[:, :],
                                 start=True, stop=True)
                r_sb = cpool.tile([128, 132], bf16, tag=f"R{kdim}")
                nc.vector.tensor_copy(out=r_sb[:kdim, 0:128],
                                      in_=r_ps[:kdim, 0:128].bitcast(bf16)[:, 0:128])
                nc.vector.tensor_copy(out=r_sb[:kdim, 128:132], in_=r_ps[:kdim, 128:132])
                return r_sb

            R1 = build_R(w1_in, att1_in, FIN)
            R2 = build_R(w2_in, att2_in, C)

            # resident state
            U_sb = cpool.tile([128, S, HC], f32)        # aggregation output per layer
            actT_sb = cpool.tile([C, S, 128], bf16)     # layer-1 activations^T (own)
            gacc = cpool.tile([128, H], f32)

            # ============ table build (replicated; all N rows) ============
            def build_table(layer):
                kdim = FIN if layer == 1 else C
                R = R1 if layer == 1 else R2
                # slab list: (src_ap_fn, row0, width)
                if layer == 1:
                    blocks = [(None, 0, N)]
                else:
                    blocks = [(k, k * NLOC, NLOC) for k in range(N_CORES)]
                for kblk, row0, width in blocks:
                    off = 0
                    while off < width:
                        w = min(G * 128, width - off)
                        ntiles_full, rem = divmod(w, 128)
                        xs = bld.tile([128, G * 128], bf16, tag="xs")
                        if layer == 1:
                            nc.sync.dma_start(out=xs[:kdim, :w],
                                              in_=xT_in[:, off : off + w])
                        else:
                            nc.sync.dma_start(
                                out=xs[:kdim, :w],
                                in_=actT_full[kblk * C : (kblk + 1) * C,
                                              off : off + w])
                        tr = bld.tile([128, G, 132], bf16, tag="tr")
                        nt = ntiles_full + (1 if rem else 0)
                        for u in range(nt):
                            rows = 128 if u < ntiles_full else rem
                            hp = bldp.tile([128, 132], f32, tag="hp", space="PSUM")
                            nc.tensor.matmul(
                                out=hp[:rows, :],
                                lhsT=xs[:kdim, u * 128 : u * 128 + rows],
                                rhs=R[:kdim, :], start=True, stop=True)
                            nc.vector.tensor_copy(out=tr[:rows, u, 0:132],
                                                  in_=hp[:rows, :])
                        r0 = row0 + off
                        if ntiles_full:
                            nc.scalar.dma_start(
                                out=table[r0 : r0 + ntiles_full * 128, 0:130]
                                    .rearrange("(u p) f -> p u f", p=128),
                                in_=tr[:, 0:ntiles_full, 0:130])
                        if rem:
                            nc.scalar.dma_start(
                                out=table[r0 + ntiles_full * 128 :
                                          r0 + ntiles_full * 128 + rem, 0:130],
                                in_=tr[:rem, ntiles_full, 0:130])
                        off += w

            # ============ adst build (own nodes) ============
            def build_adst(layer):
                kdim = FIN if layer == 1 else C
                R = R1 if layer == 1 else R2
                GA = 4
                for g0 in range(0, S, GA):
                    gn = min(GA, S - g0)
                    ap_ = bldp.tile([128, GA, 2], f32, tag="ps", space="PSUM")
                    for u in range(gn):
                        t = g0 + u
                        if layer == 1:
                            lhsT = xto[:, t * NST_NODES : (t + 1) * NST_NODES]
                        else:
                            lhsT = actT_sb[:, t, 0:NST_NODES]
                        nc.tensor.matmul(out=ap_[:NST_NODES, u, :], lhsT=lhsT,
                                         rhs=R[:kdim, 130:132], start=True, stop=True)
                    asb = bld.tile([128, GA, 2], bf16, tag="asb")
                    nc.vector.tensor_copy(out=asb[:NST_NODES, 0:gn, :],
                                          in_=ap_[:NST_NODES, 0:gn, :])
                    nc.scalar.dma_start(
                        out=adst_tbl[g0 * NST_NODES : (g0 + gn) * NST_NODES, 0:2]
                            .rearrange("(u p) f -> p u f", p=NST_NODES),
                        in_=asb[:NST_NODES, 0:gn, :])

            # ============ edge pass ============
            qctr = [0]

            def _gq():
                # Tile assigns SWDGE sem lanes per Pool-DMA call in SCHEDULED
                # order, which the scheduler may permute vs emission order - no
                # emission-time rotation can stay consistent. One queue always
                # is, and a single SWDGE queue already stripes descriptors
                # across all 16 DMA engines.
                return 0

            GMAX = int(__import__("os").environ.get("KGMAX", "8"))

            def chunked_gather(dst, blk0, nblk, src, idx_all, t, row):
                c = 0
                while c < nblk:
                    n = min(GMAX, nblk - c)
                    nc.gpsimd.dma_gather(
                        dst[:, blk0 + c : blk0 + c + n, :], src,
                        idx_all[:, t, (c * 8) : (c + n) * 8],
                        n * 128, n * 128, row, queue_num=_gq())
                    c += n

            def edge_pass(layer):
                nc.vector.memset(gacc[:], 0.0)
                for t in range(S):
                    g_t = gat.tile([128, nb, TROW], bf16, tag="G")
                    chunked_gather(g_t, 0, nb_lo, table[0:V_HALF, :], ilo_all, t, TROW)
                    chunked_gather(g_t, nb_lo, nb_hi, table[V_HALF:N, :], ihi_all, t, TROW)
                    a_t = gat.tile([128, nb, AROW], bf16, tag="A")
                    chunked_gather(a_t, 0, nb, adst_tbl[:], ia_all, t, AROW)
                    # scores: s = asrc + adst -> lrelu -> exp (+gsum accum)
                    s_t = edge.tile([128, nb, 2], f32, tag="s")
                    nc.vector.tensor_tensor(out=s_t[:], in0=g_t[:, :, HC : HC + 2],
                                            in1=a_t[:, :, 0:2], op=Alu.add)
                    neg_t = edge.tile([128, nb, 2], f32, tag="ng")
                    nc.vector.tensor_scalar(out=neg_t[:], in0=s_t[:], scalar1=0.0,
                                            scalar2=0.2, op0=Alu.min, op1=Alu.mult)
                    sl_t = edge.tile([128, nb, 2], f32, tag="sl")
                    nc.vector.scalar_tensor_tensor(out=sl_t[:], in0=s_t[:], scalar=0.0,
                                                   in1=neg_t[:], op0=Alu.max,
                                                   op1=Alu.add)
                    w_b = edge.tile([128, nb, 2, 1], bf16, tag="wb")
                    acc_t = edge.tile([128, H], f32, tag="acc")
                    for h in range(H):
                        nc.scalar.activation(
                            out=w_b[:, :, h, :], in_=sl_t[:, :, h : h + 1],
                            func=Act.Exp, accum_out=acc_t[:, h : h + 1])
                    nc.vector.tensor_tensor(out=gacc[:], in0=gacc[:], in1=acc_t[:],
                                            op=Alu.add)
                    # messages = h * w  (one op, w broadcast per head)
                    msg = edge.tile([128, nb, H, C], bf16, tag="m")
                    nc.vector.tensor_tensor(
                        out=msg[:],
                        in0=g_t[:, :, 0:HC].rearrange("p j (h c) -> p j h c", h=H),
                        in1=w_b[:].to_broadcast([128, nb, H, C]), op=Alu.mult)
                    # one-hot for all nb blocks in one op
                    oh = edge.tile([128, nb, 128], bf16, tag="oh")
                    nc.vector.tensor_tensor(
                        out=oh[:], in0=iota3[:].to_broadcast([128, nb, 128]),
                        in1=slot4[:, t, :, :].to_broadcast([128, nb, 128]),
                        op=Alu.is_equal)
                    u_ps = upool.tile([128, HC], f32, tag="U", space="PSUM")
                    for j in range(nb):
                        nc.tensor.matmul(
                            out=u_ps[:], lhsT=oh[:, j, :], rhs=msg[:, j, :, :],
                            start=(j == 0), stop=(j == nb - 1))
                    nc.scalar.activation(out=U_sb[:, t, :], in_=u_ps[:], func=Act.Copy)

            # ============ stats allreduce + fixup ============
            def stats_and_fixup(layer):
                g_ps = bldp.tile([128, H], f32, tag="ps", space="PSUM")
                nc.tensor.matmul(out=g_ps[0:1, :], lhsT=ones_col[:], rhs=gacc[:],
                                 start=True, stop=True)
                g_sb = fix.tile([1, H], f32, tag="gsb")
                nc.vector.tensor_copy(out=g_sb[:], in_=g_ps[0:1, :])
                nc.sync.dma_start(out=ar_in[:], in_=g_sb[:])
                tc.strict_bb_all_engine_barrier()
                nc.gpsimd.collective_compute(
                    "AllReduce", mybir.AluOpType.add,
                    replica_groups=[list(range(N_CORES))],
                    ins=[ar_in[:]], outs=[ar_out[:]],
                )
                tg = fix.tile([1, H], f32, tag="tg")
                nc.sync.dma_start(out=tg[:], in_=ar_out[:])
                tb_ps = bldp.tile([128, H], f32, tag="ps", space="PSUM")
                nc.tensor.matmul(out=tb_ps[:], lhsT=ones_row[0:1, :], rhs=tg[:],
                                 start=True, stop=True)
                tb = fix.tile([128, H], f32, tag="tb")
                nc.vector.tensor_scalar(out=tb[:], in0=tb_ps[:], scalar1=1.0e-10,
                                        scalar2=None, op0=Alu.add)
                rt = fix.tile([128, H], f32, tag="rt")
                nc.vector.reciprocal(out=rt[:], in_=tb[:])
                nc.vector.tensor_scalar(out=rt[:], in0=rt[:], scalar1=0.5,
                                        scalar2=None, op0=Alu.mult)

                bias = b1b if layer == 1 else b2b
                # m = U0*rt0 + U1*rt1 + bias   (batched over all supertiles)
                m1 = fix.tile([128, S, C], f32, tag="m1")
                nc.vector.tensor_scalar(out=m1[:], in0=U_sb[:, :, C:HC],
                                        scalar1=rt[:, 1:2], scalar2=None, op0=Alu.mult)
                m0 = fix.tile([128, S, C], f32, tag="m0")
                nc.vector.scalar_tensor_tensor(out=m0[:], in0=U_sb[:, :, 0:C],
                                               scalar=rt[:, 0:1], in1=m1[:],
                                               op0=Alu.mult, op1=Alu.add)
                nc.vector.tensor_tensor(out=m0[:], in0=m0[:],
                                        in1=bias[:].to_broadcast([128, S, C]),
                                        op=Alu.add)
                if layer == 1:
                    act_n = fix.tile([128, S, C], bf16, tag="an")
                    nc.scalar.activation(out=act_n[:], in_=m0[:], func=Act.Relu)
                    for t in range(S):
                        tp = bldp.tile([64, 128], f32, tag="ps", space="PSUM")
                        nc.tensor.transpose(
                            out=tp[:, 0:128].bitcast(bf16)[:, 0:NST_NODES],
                            in_=act_n[0:NST_NODES, t, :],
                            identity=ident[:NST_NODES, :NST_NODES])
                        nc.scalar.activation(
                            out=actT_sb[:, t, 0:NST_NODES],
                            in_=tp[:, 0:128].bitcast(bf16)[:, 0:NST_NODES],
                            func=Act.Copy)
                    nc.sync.dma_start(
                        out=ag_in[:, :].rearrange("c (t p) -> c t p", t=S),
                        in_=actT_sb[:, :, 0:NST_NODES])
                else:
                    nc.sync.dma_start(
                        out=out_ext[:, :].rearrange("(t p) c -> p t c", p=NST_NODES),
                        in_=m0[0:NST_NODES, :, :])

            # ============ main sequence ============
            if phases >= 1:
                build_table(1)
                build_adst(1)
            if phases >= 2:
                tc.strict_bb_all_engine_barrier()
                edge_pass(1)
            if phases >= 3:
                stats_and_fixup(1)
            if phases >= 4:
                tc.strict_bb_all_engine_barrier()
                nc.gpsimd.collective_compute(
                    "AllGather", mybir.AluOpType.bypass,
                    replica_groups=[list(range(N_CORES))],
                    ins=[ag_in[:]], outs=[actT_full[:]],
                )
            if phases >= 5:
                tc.strict_bb_all_engine_barrier()
                build_table(2)
                build_adst(2)
            if phases >= 6:
                tc.strict_bb_all_engine_barrier()
                edge_pass(2)
            if phases >= 7:
                stats_and_fixup(2)

    nc.compile()
    return nc


# --------------------------------------------------------------------------
# entry point
# --------------------------------------------------------------------------

def _make_in_maps(x, edge_index, W1, att1, b1, W2, att2, b2):
    import ml_dtypes

    x = np.asarray(x, np.float32)
    xT = np.ascontiguousarray(x.T).astype(ml_dtypes.bfloat16)
    ilo, ihi, ia, slot, nb_lo, nb_hi = _preprocess(np.asarray(edge_index))
    common = {
        "xT": xT,
        "W1": np.asarray(W1, np.float32), "W2": np.asarray(W2, np.float32),
        "att1": np.asarray(att1, np.float32), "att2": np.asarray(att2, np.float32),
        "b1": np.asarray(b1, np.float32), "b2": np.asarray(b2, np.float32),
    }
    in_maps = []
    for k in range(N_CORES):
        m = dict(common)
        m["xTo"] = np.ascontiguousarray(xT[:, k * NLOC : (k + 1) * NLOC])
        m["ilo"], m["ihi"], m["ia"], m["slot"] = ilo[k], ihi[k], ia[k], slot[k]
        in_maps.append(m)
    return in_maps, (nb_lo, nb_hi)


def kernel(x, edge_index, W1, att1, b1, W2, att2, b2):
    global _compiled
    from concourse.bass_utils import run_bass_kernel_spmd

    in_maps, key = _make_in_maps(x, edge_index, W1, att1, b1, W2, att2, b2)
    if _compiled is None or _compiled[1] != key:
        nc = _build_program(*key)
        _compiled = (nc, key)
    nc = _compiled[0]

    res = run_bass_kernel_spmd(nc, in_maps, list(range(N_CORES)))
    out = np.concatenate([res.results[k]["out"] for k in range(N_CORES)], axis=0)
    return out


# revision 12
# speedup vs baseline: 1.6985x; 1.6985x over previous
"""GAT (2-layer, 2-head, global-softmax) Trainium2 kernel over 8 NeuronCores.

Strategy: nodes partitioned by destination across 8 cores (6250 dst each);
edges live with their dst owner. The GLOBAL softmax factorizes: out =
[sum_e exp(s_e) h_src] / gsum with gsum a per-head global scalar, so each
layer is ONE pass over edges plus a tiny AllReduce (max-subtraction dropped;
it only guards a 1e-10 epsilon that is ~1e-15 relative here).

Layout/engine assignment (v2, rebuilt from NTFF profile of v1):
  - table [N, 256] bf16 rows (512B, gather granularity), only cols 0:130
    written (h 0:128, asrc 128:130) -> table-write traffic halved.
  - adst table bf16 [NLOC+1, 128] (256B rows, gather min) with -1e9 sentinel.
  - build phases: host passes x^T in bf16; lhsT slabs of 8 tiles per DMA
    (sync-engine issue count was the v1 build bottleneck); stores issued on
    the Activation engine's HWDGE queue; PSUM->SBUF copies fused (h|asrc in
    one copy).
  - edge pass per supertile: 3 dma_gather calls (994ns fixed cost each was
    dominant at v1's 7 calls), one batched is_equal builds all nb one-hot
    blocks (v1: 20 separate ops), lrelu/exp/U-copy on the Scalar engine,
    message multiply as one broadcast op. Segment-sum via one-hot matmul
    accumulation in PSUM (unchanged).
  - layer-1 activations are transposed at fixup (50 PE transposes) and
    AllGathered as [C, NLOC] so layer-2 table build needs no transposes.
"""
import sys

sys.path.insert(0, "/opt/trn_rl_repo")

import numpy as np

N = 50000
FIN = 128
C = 64
H = 2
HC = H * C  # 128
E = 800000
N_CORES = 8
NLOC = N // N_CORES          # 6250
NST_NODES = 125              # dst nodes per supertile
S = NLOC // NST_NODES        # 50 supertiles per core
S_GLOBAL = S * N_CORES       # 400
V_HALF = N // 2              # 25000 rows per table half (int16-safe indices)
TROW = 256                   # bf16 elems per table row (512 B): h[0:128], asrc[128:130]
AROW = 128                   # bf16 elems per adst row (256 B): adst[0:2]
SENT_A = NLOC                # adst sentinel row (-1e9)

_compiled = None  # (nc, (nb_lo, nb_hi))


# --------------------------------------------------------------------------
# host-side graph preprocessing (pure index manipulation)
# --------------------------------------------------------------------------

def _wrap_idx(flat):
    """[n] -> [128, n/16] int16 wrapped + 8x replicated layout for dma_gather."""
    w = np.asarray(flat, np.int16).reshape(-1, 16).T
    return np.tile(w, (8, 1))


def _preprocess(edge_index):
    src = np.concatenate([edge_index[0].astype(np.int64), np.arange(N, dtype=np.int64)])
    dst = np.concatenate([edge_index[1].astype(np.int64), np.arange(N, dtype=np.int64)])
    order = np.argsort(dst, kind="stable")
    src, dst = src[order], dst[order]

    stg = (dst // NST_NODES).astype(np.int64)          # global supertile id, sorted
    starts = np.searchsorted(stg, np.arange(S_GLOBAL))
    ends = np.searchsorted(stg, np.arange(S_GLOBAL), side="right")
    lo_mask = src < V_HALF
    n_lo = np.array([int(lo_mask[a:b].sum()) for a, b in zip(starts, ends)])
    n_hi = (ends - starts) - n_lo
    nb_lo = int(np.ceil(n_lo.max() / 128))
    nb_hi = int(np.ceil(n_hi.max() / 128))
    cap_lo, cap_hi = nb_lo * 128, nb_hi * 128
    nb = nb_lo + nb_hi
    cap = cap_lo + cap_hi

    ilo = np.zeros((N_CORES, S, 128, cap_lo // 16), np.int16)
    ihi = np.zeros((N_CORES, S, 128, cap_hi // 16), np.int16)
    ia = np.zeros((N_CORES, S, 128, cap // 16), np.int16)
    slot = np.zeros((N_CORES, S, 128, nb), np.float32)

    for g in range(S_GLOBAL):
        k, t = divmod(g, S)
        a, b = starts[g], ends[g]
        s_src, s_dst = src[a:b], dst[a:b]
        m = s_src < V_HALF
        src_lo, dst_lo = s_src[m], s_dst[m]
        src_hi, dst_hi = s_src[~m], s_dst[~m]

        i_lo = np.zeros(cap_lo, np.int64)
        i_lo[: len(src_lo)] = src_lo
        i_hi = np.zeros(cap_hi, np.int64)
        i_hi[: len(src_hi)] = src_hi - V_HALF

        d_all = np.full(cap, SENT_A, np.int64)
        d_all[: len(dst_lo)] = dst_lo - NLOC * k
        d_all[cap_lo : cap_lo + len(dst_hi)] = dst_hi - NLOC * k

        sl = np.full(cap, 127, np.int64)
        sl[: len(dst_lo)] = dst_lo - NST_NODES * g
        sl[cap_lo : cap_lo + len(dst_hi)] = dst_hi - NST_NODES * g

        ilo[k, t] = _wrap_idx(i_lo)
        ihi[k, t] = _wrap_idx(i_hi)
        ia[k, t] = _wrap_idx(d_all)
        slot[k, t] = sl.reshape(nb, 128).T.astype(np.float32)

    return ilo, ihi, ia, slot, nb_lo, nb_hi


# --------------------------------------------------------------------------
# device program
# --------------------------------------------------------------------------

def _build_program(nb_lo, nb_hi, phases=7):
    import concourse.bass as bass
    import concourse.bacc as bacc
    import concourse.mybir as mybir
    from concourse import library_config
    from concourse.masks import make_identity
    from concourse.tile import TileContext

    f32 = mybir.dt.float32
    bf16 = mybir.dt.bfloat16
    i16 = mybir.dt.int16
    i32 = mybir.dt.int32
    Alu = mybir.AluOpType
    Act = mybir.ActivationFunctionType

    nb = nb_lo + nb_hi
    cap_lo, cap_hi, cap = nb_lo * 128, nb_hi * 128, (nb_lo + nb_hi) * 128

    nc = bacc.Bacc("TRN2", target_bir_lowering=False, debug=False,
                   num_devices=N_CORES, num_swdge_queues=4)

    # ---- I/O
    xT_in = nc.declare_dram_parameter("xT", [FIN, N], bf16, isOutput=False)
    xTo_in = nc.declare_dram_parameter("xTo", [FIN, NLOC], bf16, isOutput=False)
    w1_in = nc.declare_dram_parameter("W1", [HC, FIN], f32, isOutput=False)
    w2_in = nc.declare_dram_parameter("W2", [HC, C], f32, isOutput=False)
    att1_in = nc.declare_dram_parameter("att1", [1, H, 2 * C], f32, isOutput=False)
    att2_in = nc.declare_dram_parameter("att2", [1, H, 2 * C], f32, isOutput=False)
    b1_in = nc.declare_dram_parameter("b1", [C], f32, isOutput=False)
    b2_in = nc.declare_dram_parameter("b2", [C], f32, isOutput=False)
    ilo_in = nc.declare_dram_parameter("ilo", [S, 128, cap_lo // 16], i16, isOutput=False)
    ihi_in = nc.declare_dram_parameter("ihi", [S, 128, cap_hi // 16], i16, isOutput=False)
    ia_in = nc.declare_dram_parameter("ia", [S, 128, cap // 16], i16, isOutput=False)
    slot_in = nc.declare_dram_parameter("slot", [S, 128, nb], f32, isOutput=False)
    out_ext = nc.declare_dram_parameter("out", [NLOC, C], f32, isOutput=True)

    # ---- internal DRAM
    table = nc.dram_tensor("table_d", [N, TROW], bf16)
    adst_tbl = nc.dram_tensor("adst_d", [NLOC + 1, AROW], bf16)
    ar_in = nc.dram_tensor("ar_in_d", [1, H], f32)
    ar_out = nc.dram_tensor("ar_out_d", [1, H], f32, addr_space="Shared")
    ag_in = nc.dram_tensor("ag_in_d", [C, NLOC], bf16)
    actT_full = nc.dram_tensor("actT_full_d", [N_CORES * C, NLOC], bf16, addr_space="Shared")

    G = 8  # tiles per build slab

    with TileContext(nc) as tc:
        with (
            tc.tile_pool(name="const", bufs=1) as cpool,
            tc.tile_pool(name="bld", bufs=3) as bld,
            tc.tile_pool(name="bldp", bufs=2, space="PSUM") as bldp,
            tc.tile_pool(name="gat", bufs=2) as gat,
            tc.tile_pool(name="edge", bufs=2) as edge,
            tc.tile_pool(name="up", bufs=2, space="PSUM") as upool,
            tc.tile_pool(name="fix", bufs=1) as fix,
        ):
            nc.gpsimd.load_library(library_config.mlp)

            # ============ constants ============
            ident = cpool.tile([128, 128], bf16)
            make_identity(nc, ident[:])
            iota_i = cpool.tile([128, 128], i32)
            nc.gpsimd.iota(iota_i[:], pattern=[[1, 128]], base=0, channel_multiplier=0)
            iota3 = cpool.tile([128, 1, 128], bf16)
            nc.vector.tensor_copy(out=iota3[:, 0, :], in_=iota_i[:])
            ones_row = cpool.tile([1, 128], f32)
            nc.vector.memset(ones_row[:], 1.0)
            ones_col = cpool.tile([128, 1], f32)
            nc.vector.memset(ones_col[:], 1.0)

            # all idx/slot arrays, resident (layer-independent)
            ilo_all = cpool.tile([128, S, cap_lo // 16], i16)
            ihi_all = cpool.tile([128, S, cap_hi // 16], i16)
            ia_all = cpool.tile([128, S, cap // 16], i16)
            nc.sync.dma_start(out=ilo_all[:], in_=ilo_in.ap().rearrange("s p w -> p s w"))
            nc.sync.dma_start(out=ihi_all[:], in_=ihi_in.ap().rearrange("s p w -> p s w"))
            nc.sync.dma_start(out=ia_all[:], in_=ia_in.ap().rearrange("s p w -> p s w"))
            slot4 = cpool.tile([128, S, nb, 1], f32)
            nc.sync.dma_start(
                out=slot4[:, :, :, 0], in_=slot_in.ap().rearrange("s p w -> p s w")
            )

            # own-node x^T, resident (adst layer-1 lhsT)
            xto = cpool.tile([128, NLOC], bf16)
            nc.sync.dma_start(out=xto[:], in_=xTo_in[:, :])

            # bias broadcast tiles [128, 1, C]
            b1b = cpool.tile([128, 1, C], f32)
            nc.sync.dma_start(out=b1b[:, 0, :], in_=b1_in.ap().partition_broadcast(128))
            b2b = cpool.tile([128, 1, C], f32)
            nc.sync.dma_start(out=b2b[:, 0, :], in_=b2_in.ap().partition_broadcast(128))

            # sentinel row of adst table
            sent_t = cpool.tile([1, AROW], bf16)
            nc.vector.memset(sent_t[:], -1.0e9)
            nc.sync.dma_start(out=adst_tbl[SENT_A : SENT_A + 1, :], in_=sent_t[:])

            # ---- R matrices: R = [W^T | Vsrc' | Vdst']  (bf16, [K=kdim, 132])
            def build_R(w_dram, att_dram, kdim):
                wt = bld.tile([128, kdim], f32, tag="wld")
                nc.sync.dma_start(out=wt[:, :], in_=w_dram[:])          # [HC, kdim]
                wb = bld.tile([128, kdim], bf16, tag="wldb")
                nc.vector.tensor_copy(out=wb[:], in_=wt[:])
                vsd = cpool.tile([128, 4], f32, tag="vsd")  # cols 0:2 Vsrc, 2:4 Vdst
                nc.vector.memset(vsd[:], 0.0)
                for h in range(H):
                    nc.sync.dma_start(
                        out=vsd[h * C : (h + 1) * C, h : h + 1],
                        in_=att_dram[0:1, h, C : 2 * C].rearrange("o c -> c o"),
                    )
                    nc.sync.dma_start(
                        out=vsd[h * C : (h + 1) * C, 2 + h : 3 + h],
                        in_=att_dram[0:1, h, 0:C].rearrange("o c -> c o"),
                    )
                vsdb = cpool.tile([128, 4], bf16, tag="vsdb")
                nc.vector.tensor_copy(out=vsdb[:], in_=vsd[:])
                r_ps = bldp.tile([128, 132], f32, tag="ps", space="PSUM")
                nc.tensor.transpose(
                    out=r_ps[:kdim, 0:128].bitcast(bf16)[:, 0:128], in_=wb[:, :],
                    identity=ident[:],
                )
                nc.tensor.matmul(out=r_ps[:kdim, 128:132], lhsT=wb[:, :], rhs=vsdb[:, :],
                                 start=True, stop=True)
                r_sb = cpool.tile([128, 132], bf16, tag=f"R{kdim}")
                nc.vector.tensor_copy(out=r_sb[:kdim, 0:128],
                                      in_=r_ps[:kdim, 0:128].bitcast(bf16)[:, 0:128])
                nc.vector.tensor_copy(out=r_sb[:kdim, 128:132], in_=r_ps[:kdim, 128:132])
                return r_sb

            R1 = build_R(w1_in, att1_in, FIN)
            R2 = build_R(w2_in, att2_in, C)

            # resident state
            U_sb = cpool.tile([128, S, HC], f32)        # aggregation output per layer
            actT_sb = cpool.tile([C, S, 128], bf16)     # layer-1 activations^T (own)
            gacc = cpool.tile([128, H], f32)

            # ============ table build (replicated; all N rows) ============
            def build_table(layer):
                kdim = FIN if layer == 1 else C
                R = R1 if layer == 1 else R2
                # slab list: (src_ap_fn, row0, width)
                if layer == 1:
                    blocks = [(None, 0, N)]
                else:
                    blocks = [(k, k * NLOC, NLOC) for k in range(N_CORES)]
                for kblk, row0, width in blocks:
                    off = 0
                    while off < width:
                        w = min(G * 128, width - off)
                        ntiles_full, rem = divmod(w, 128)
                        xs = bld.tile([128, G * 128], bf16, tag="xs")
                        if layer == 1:
                            nc.sync.dma_start(out=xs[:kdim, :w],
                                              in_=xT_in[:, off : off + w])
                        else:
                            nc.sync.dma_start(
                                out=xs[:kdim, :w],
                                in_=actT_full[kblk * C : (kblk + 1) * C,
                                              off : off + w])
                        tr = bld.tile([128, G, 132], bf16, tag="tr")
                        nt = ntiles_full + (1 if rem else 0)
                        for u in range(nt):
                            rows = 128 if u < ntiles_full else rem
                            hp = bldp.tile([128, 132], f32, tag="hp", space="PSUM")
                            nc.tensor.matmul(
                                out=hp[:rows, :],
                                lhsT=xs[:kdim, u * 128 : u * 128 + rows],
                                rhs=R[:kdim, :], start=True, stop=True)
                            nc.vector.tensor_copy(out=tr[:rows, u, 0:132],
                                                  in_=hp[:rows, :])
                        r0 = row0 + off
                        if ntiles_full:
                            nc.scalar.dma_start(
                                out=table[r0 : r0 + ntiles_full * 128, 0:130]
                                    .rearrange("(u p) f -> p u f", p=128),
                                in_=tr[:, 0:ntiles_full, 0:130])
                        if rem:
                            nc.scalar.dma_start(
                                out=table[r0 + ntiles_full * 128 :
                                          r0 + ntiles_full * 128 + rem, 0:130],
                                in_=tr[:rem, ntiles_full, 0:130])
                        off += w

            # ============ adst build (own nodes) ============
            def build_adst(layer):
                kdim = FIN if layer == 1 else C
                R = R1 if layer == 1 else R2
                GA = 4
                for g0 in range(0, S, GA):
                    gn = min(GA, S - g0)
                    ap_ = bldp.tile([128, GA, 2], f32, tag="ps", space="PSUM")
                    for u in range(gn):
                        t = g0 + u
                        if layer == 1:
                            lhsT = xto[:, t * NST_NODES : (t + 1) * NST_NODES]
                        else:
                            lhsT = actT_sb[:, t, 0:NST_NODES]
                        nc.tensor.matmul(out=ap_[:NST_NODES, u, :], lhsT=lhsT,
                                         rhs=R[:kdim, 130:132], start=True, stop=True)
                    asb = bld.tile([128, GA, 2], bf16, tag="asb")
                    nc.vector.tensor_copy(out=asb[:NST_NODES, 0:gn, :],
                                          in_=ap_[:NST_NODES, 0:gn, :])
                    nc.scalar.dma_start(
                        out=adst_tbl[g0 * NST_NODES : (g0 + gn) * NST_NODES, 0:2]
                            .rearrange("(u p) f -> p u f", p=NST_NODES),
                        in_=asb[:NST_NODES, 0:gn, :])

            # ============ edge pass ============
            qctr = [0]

            def _gq():
                # Tile assigns SWDGE sem lanes per Pool-DMA call in SCHEDULED
                # order, which the scheduler may permute vs emission order - no
                # emission-time rotation can stay consistent. One queue always
                # is, and a single SWDGE queue already stripes descriptors
                # across all 16 DMA engines.
                return 0

            GMAX = int(__import__("os").environ.get("KGMAX", "8"))

            def chunked_gather(dst, blk0, nblk, src, idx_all, t, row):
                c = 0
                while c < nblk:
                    n = min(GMAX, nblk - c)
                    nc.gpsimd.dma_gather(
                        dst[:, blk0 + c : blk0 + c + n, :], src,
                        idx_all[:, t, (c * 8) : (c + n) * 8],
                        n * 128, n * 128, row, queue_num=_gq())
                    c += n

            def edge_pass(layer):
                nc.vector.memset(gacc[:], 0.0)
                for t in range(S):
                    g_t = gat.tile([128, nb, TROW], bf16, tag="G")
                    chunked_gather(g_t, 0, nb_lo, table[0:V_HALF, :], ilo_all, t, TROW)
                    chunked_gather(g_t, nb_lo, nb_hi, table[V_HALF:N, :], ihi_all, t, TROW)
                    a_t = gat.tile([128, nb, AROW], bf16, tag="A")
                    chunked_gather(a_t, 0, nb, adst_tbl[:], ia_all, t, AROW)
                    # scores: s = asrc + adst -> lrelu -> exp (+gsum accum)
                    s_t = edge.tile([128, nb, 2], f32, tag="s")
                    nc.vector.tensor_tensor(out=s_t[:], in0=g_t[:, :, HC : HC + 2],
                                            in1=a_t[:, :, 0:2], op=Alu.add)
                    neg_t = edge.tile([128, nb, 2], f32, tag="ng")
                    nc.vector.tensor_scalar(out=neg_t[:], in0=s_t[:], scalar1=0.0,
                                            scalar2=0.2, op0=Alu.min, op1=Alu.mult)
                    sl_t = edge.tile([128, nb, 2], f32, tag="sl")
                    nc.vector.scalar_tensor_tensor(out=sl_t[:], in0=s_t[:], scalar=0.0,
                                                   in1=neg_t[:], op0=Alu.max,
                                                   op1=Alu.add)
                    w_b = edge.tile([128, nb, 2, 1], bf16, tag="wb")
                    acc_t = edge.tile([128, H], f32, tag="acc")
                    for h in range(H):
                        nc.scalar.activation(
                            out=w_b[:, :, h, :], in_=sl_t[:, :, h : h + 1],
                            func=Act.Exp, accum_out=acc_t[:, h : h + 1])
                    nc.vector.tensor_tensor(out=gacc[:], in0=gacc[:], in1=acc_t[:],
                                            op=Alu.add)
                    # messages = h * w  (one op, w broadcast per head)
                    msg = edge.tile([128, nb, H, C], bf16, tag="m")
                    nc.vector.tensor_tensor(
                        out=msg[:],
                        in0=g_t[:, :, 0:HC].rearrange("p j (h c) -> p j h c", h=H),
                        in1=w_b[:].to_broadcast([128, nb, H, C]), op=Alu.mult)
                    # one-hot for all nb blocks in one op
                    oh = edge.tile([128, nb, 128], bf16, tag="oh")
                    nc.vector.tensor_tensor(
                        out=oh[:], in0=iota3[:].to_broadcast([128, nb, 128]),
                        in1=slot4[:, t, :, :].to_broadcast([128, nb, 128]),
                        op=Alu.is_equal)
                    u_ps = upool.tile([128, HC], f32, tag="U", space="PSUM")
                    for j in range(nb):
                        nc.tensor.matmul(
                            out=u_ps[:], lhsT=oh[:, j, :], rhs=msg[:, j, :, :],
                            start=(j == 0), stop=(j == nb - 1))
                    nc.scalar.activation(out=U_sb[:, t, :], in_=u_ps[:], func=Act.Copy)

            # ============ stats allreduce + fixup ============
            def stats_and_fixup(layer):
                g_ps = bldp.tile([128, H], f32, tag="ps", space="PSUM")
                nc.tensor.matmul(out=g_ps[0:1, :], lhsT=ones_col[:], rhs=gacc[:],
                                 start=True, stop=True)
                g_sb = fix.tile([1, H], f32, tag="gsb")
                nc.vector.tensor_copy(out=g_sb[:], in_=g_ps[0:1, :])
                nc.sync.dma_start(out=ar_in[:], in_=g_sb[:])
                tc.strict_bb_all_engine_barrier()
                nc.gpsimd.collective_compute(
                    "AllReduce", mybir.AluOpType.add,
                    replica_groups=[list(range(N_CORES))],
                    ins=[ar_in[:]], outs=[ar_out[:]],
                )
                tg = fix.tile([1, H], f32, tag="tg")
                nc.sync.dma_start(out=tg[:], in_=ar_out[:])
                tb_ps = bldp.tile([128, H], f32, tag="ps", space="PSUM")
                nc.tensor.matmul(out=tb_ps[:], lhsT=ones_row[0:1, :], rhs=tg[:],
                                 start=True, stop=True)
                tb = fix.tile([128, H], f32, tag="tb")
                nc.vector.tensor_scalar(out=tb[:], in0=tb_ps[:], scalar1=1.0e-10,
                                        scalar2=None, op0=Alu.add)
                rt = fix.tile([128, H], f32, tag="rt")
                nc.vector.reciprocal(out=rt[:], in_=tb[:])
                nc.vector.tensor_scalar(out=rt[:], in0=rt[:], scalar1=0.5,
                                        scalar2=None, op0=Alu.mult)

                bias = b1b if layer == 1 else b2b
                # m = U0*rt0 + U1*rt1 + bias   (batched over all supertiles)
                m1 = fix.tile([128, S, C], f32, tag="m1")
                nc.vector.tensor_scalar(out=m1[:], in0=U_sb[:, :, C:HC],
                                        scalar1=rt[:, 1:2], scalar2=None, op0=Alu.mult)
                m0 = fix.tile([128, S, C], f32, tag="m0")
                nc.vector.scalar_tensor_tensor(out=m0[:], in0=U_sb[:, :, 0:C],
                                               scalar=rt[:, 0:1], in1=m1[:],
                                               op0=Alu.mult, op1=Alu.add)
                nc.vector.tensor_tensor(out=m0[:], in0=m0[:],
                                        in1=bias[:].to_broadcast([128, S, C]),
                                        op=Alu.add)
                if layer == 1:
                    act_n = fix.tile([128, S, C], bf16, tag="an")
                    nc.scalar.activation(out=act_n[:], in_=m0[:], func=Act.Relu)
                    for t in range(S):
                        tp = bldp.tile([64, 128], f32, tag="ps", space="PSUM")
                        nc.tensor.transpose(
                            out=tp[:, 0:128].bitcast(bf16)[:, 0:NST_NODES],
                            in_=act_n[0:NST_NODES, t, :],
                            identity=ident[:NST_NODES, :NST_NODES])
                        nc.scalar.activation(
                            out=actT_sb[:, t, 0:NST_NODES],
                            in_=tp[:, 0:128].bitcast(bf16)[:, 0:NST_NODES],
                            func=Act.Copy)
                    nc.sync.dma_start(
                        out=ag_in[:, :].rearrange("c (t p) -> c t p", t=S),
                        in_=actT_sb[:, :, 0:NST_NODES])
                else:
                    nc.sync.dma_start(
                        out=out_ext[:, :].rearrange("(t p) c -> p t c", p=NST_NODES),
                        in_=m0[0:NST_NODES, :, :])

            # ============ main sequence ============
            if phases >= 1:
                build_table(1)
                build_adst(1)
            if phases >= 2:
                tc.strict_bb_all_engine_barrier()
                edge_pass(1)
            if phases >= 3:
                stats_and_fixup(1)
            if phases >= 4:
                tc.strict_bb_all_engine_barrier()
                nc.gpsimd.collective_compute(
                    "AllGather", mybir.AluOpType.bypass,
                    replica_groups=[list(range(N_CORES))],
                    ins=[ag_in[:]], outs=[actT_full[:]],
                )
            if phases >= 5:
                tc.strict_bb_all_engine_barrier()
                build_table(2)
                build_adst(2)
            if phases >= 6:
                tc.strict_bb_all_engine_barrier()
                edge_pass(2)
            if phases >= 7:
                stats_and_fixup(2)

    # Re-assign SWDGE queues in SCHEDULED order: Tile gives Pool-DMA call i the
    # DMASW sem lane i%8 (scheduled order), and a lane's sem may only ever fire
    # from one queue -> queue must be i%4 in the same order. Emission-time
    # assignment can't know the schedule, so rewrite after scheduling.
    idx = 0
    for bb in nc.m.functions[0].blocks:
        for inst in bb.instructions:
            if isinstance(inst, mybir.InstDMAGatherAnt):
                inst.queue_num = idx % 4
                idx += 1

    nc.compile()
    return nc


# --------------------------------------------------------------------------
# entry point
# --------------------------------------------------------------------------

def _make_in_maps(x, edge_index, W1, att1, b1, W2, att2, b2):
    import ml_dtypes

    x = np.asarray(x, np.float32)
    xT = np.ascontiguousarray(x.T).astype(ml_dtypes.bfloat16)
    ilo, ihi, ia, slot, nb_lo, nb_hi = _preprocess(np.asarray(edge_index))
    common = {
        "xT": xT,
        "W1": np.asarray(W1, np.float32), "W2": np.asarray(W2, np.float32),
        "att1": np.asarray(att1, np.float32), "att2": np.asarray(att2, np.float32),
        "b1": np.asarray(b1, np.float32), "b2": np.asarray(b2, np.float32),
    }
    in_maps = []
    for k in range(N_CORES):
        m = dict(common)
        m["xTo"] = np.ascontiguousarray(xT[:, k * NLOC : (k + 1) * NLOC])
        m["ilo"], m["ihi"], m["ia"], m["slot"] = ilo[k], ihi[k], ia[k], slot[k]
        in_maps.append(m)
    return in_maps, (nb_lo, nb_hi)


def kernel(x, edge_index, W1, att1, b1, W2, att2, b2):
    global _compiled
    from concourse.bass_utils import run_bass_kernel_spmd

    in_maps, key = _make_in_maps(x, edge_index, W1, att1, b1, W2, att2, b2)
    if _compiled is None or _compiled[1] != key:
        nc = _build_program(*key)
        _compiled = (nc, key)
    nc = _compiled[0]

    res = run_bass_kernel_spmd(nc, in_maps, list(range(N_CORES)))
    out = np.concatenate([res.results[k]["out"] for k in range(N_CORES)], axis=0)
    return out


# revision 13
# speedup vs baseline: 1.7360x; 1.0221x over previous
"""GAT (2-layer, 2-head, global-softmax) Trainium2 kernel over 8 NeuronCores.

Strategy: nodes partitioned by destination across 8 cores (6250 dst each);
edges live with their dst owner. The GLOBAL softmax factorizes: out =
[sum_e exp(s_e) h_src] / gsum with gsum a per-head global scalar, so each
layer is ONE pass over edges plus a tiny AllReduce (max-subtraction dropped;
it only guards a 1e-10 epsilon that is ~1e-15 relative here).

Layout/engine assignment (v2, rebuilt from NTFF profile of v1):
  - table [N, 256] bf16 rows (512B, gather granularity), only cols 0:130
    written (h 0:128, asrc 128:130) -> table-write traffic halved.
  - adst table bf16 [NLOC+1, 128] (256B rows, gather min) with -1e9 sentinel.
  - build phases: host passes x^T in bf16; lhsT slabs of 8 tiles per DMA
    (sync-engine issue count was the v1 build bottleneck); stores issued on
    the Activation engine's HWDGE queue; PSUM->SBUF copies fused (h|asrc in
    one copy).
  - edge pass per supertile: 3 dma_gather calls (994ns fixed cost each was
    dominant at v1's 7 calls), one batched is_equal builds all nb one-hot
    blocks (v1: 20 separate ops), lrelu/exp/U-copy on the Scalar engine,
    message multiply as one broadcast op. Segment-sum via one-hot matmul
    accumulation in PSUM (unchanged).
  - layer-1 activations are transposed at fixup (50 PE transposes) and
    AllGathered as [C, NLOC] so layer-2 table build needs no transposes.
"""
import sys

sys.path.insert(0, "/opt/trn_rl_repo")

import numpy as np

N = 50000
FIN = 128
C = 64
H = 2
HC = H * C  # 128
E = 800000
N_CORES = 8
NLOC = N // N_CORES          # 6250
NST_NODES = 125              # dst nodes per supertile
S = NLOC // NST_NODES        # 50 supertiles per core
S_GLOBAL = S * N_CORES       # 400
V_HALF = N // 2              # 25000 rows per table half (int16-safe indices)
TROW = 256                   # bf16 elems per table row (512 B): h[0:128], asrc[128:130]
AROW = 128                   # bf16 elems per adst row (256 B): adst[0:2]
SENT_A = NLOC                # adst sentinel row (-1e9)

_compiled = None  # (nc, (nb_lo, nb_hi))


# --------------------------------------------------------------------------
# host-side graph preprocessing (pure index manipulation)
# --------------------------------------------------------------------------

def _wrap_idx(flat):
    """[n] -> [128, n/16] int16 wrapped + 8x replicated layout for dma_gather."""
    w = np.asarray(flat, np.int16).reshape(-1, 16).T
    return np.tile(w, (8, 1))


def _preprocess(edge_index):
    src = np.concatenate([edge_index[0].astype(np.int64), np.arange(N, dtype=np.int64)])
    dst = np.concatenate([edge_index[1].astype(np.int64), np.arange(N, dtype=np.int64)])
    order = np.argsort(dst, kind="stable")
    src, dst = src[order], dst[order]

    stg = (dst // NST_NODES).astype(np.int64)          # global supertile id, sorted
    starts = np.searchsorted(stg, np.arange(S_GLOBAL))
    ends = np.searchsorted(stg, np.arange(S_GLOBAL), side="right")
    lo_mask = src < V_HALF
    n_lo = np.array([int(lo_mask[a:b].sum()) for a, b in zip(starts, ends)])
    n_hi = (ends - starts) - n_lo
    nb_lo = int(np.ceil(n_lo.max() / 128))
    nb_hi = int(np.ceil(n_hi.max() / 128))
    cap_lo, cap_hi = nb_lo * 128, nb_hi * 128
    nb = nb_lo + nb_hi
    cap = cap_lo + cap_hi

    ilo = np.zeros((N_CORES, S, 128, cap_lo // 16), np.int16)
    ihi = np.zeros((N_CORES, S, 128, cap_hi // 16), np.int16)
    ia = np.zeros((N_CORES, S, 128, cap // 16), np.int16)
    slot = np.zeros((N_CORES, S, 128, nb), np.float32)

    for g in range(S_GLOBAL):
        k, t = divmod(g, S)
        a, b = starts[g], ends[g]
        s_src, s_dst = src[a:b], dst[a:b]
        m = s_src < V_HALF
        src_lo, dst_lo = s_src[m], s_dst[m]
        src_hi, dst_hi = s_src[~m], s_dst[~m]

        i_lo = np.zeros(cap_lo, np.int64)
        i_lo[: len(src_lo)] = src_lo
        i_hi = np.zeros(cap_hi, np.int64)
        i_hi[: len(src_hi)] = src_hi - V_HALF

        d_all = np.full(cap, SENT_A, np.int64)
        d_all[: len(dst_lo)] = dst_lo - NLOC * k
        d_all[cap_lo : cap_lo + len(dst_hi)] = dst_hi - NLOC * k

        sl = np.full(cap, 127, np.int64)
        sl[: len(dst_lo)] = dst_lo - NST_NODES * g
        sl[cap_lo : cap_lo + len(dst_hi)] = dst_hi - NST_NODES * g

        ilo[k, t] = _wrap_idx(i_lo)
        ihi[k, t] = _wrap_idx(i_hi)
        ia[k, t] = _wrap_idx(d_all)
        slot[k, t] = sl.reshape(nb, 128).T.astype(np.float32)

    return ilo, ihi, ia, slot, nb_lo, nb_hi


# --------------------------------------------------------------------------
# device program
# --------------------------------------------------------------------------

def _build_program(nb_lo, nb_hi, phases=7):
    import concourse.bass as bass
    import concourse.bacc as bacc
    import concourse.mybir as mybir
    from concourse import library_config
    from concourse.masks import make_identity
    from concourse.tile import TileContext

    f32 = mybir.dt.float32
    bf16 = mybir.dt.bfloat16
    i16 = mybir.dt.int16
    i32 = mybir.dt.int32
    Alu = mybir.AluOpType
    Act = mybir.ActivationFunctionType

    nb = nb_lo + nb_hi
    cap_lo, cap_hi, cap = nb_lo * 128, nb_hi * 128, (nb_lo + nb_hi) * 128

    nc = bacc.Bacc("TRN2", target_bir_lowering=False, debug=False,
                   num_devices=N_CORES, num_swdge_queues=4)

    # ---- I/O
    xT_in = nc.declare_dram_parameter("xT", [FIN, N], bf16, isOutput=False)
    xTo_in = nc.declare_dram_parameter("xTo", [FIN, NLOC], bf16, isOutput=False)
    w1_in = nc.declare_dram_parameter("W1", [HC, FIN], f32, isOutput=False)
    w2_in = nc.declare_dram_parameter("W2", [HC, C], f32, isOutput=False)
    att1_in = nc.declare_dram_parameter("att1", [1, H, 2 * C], f32, isOutput=False)
    att2_in = nc.declare_dram_parameter("att2", [1, H, 2 * C], f32, isOutput=False)
    b1_in = nc.declare_dram_parameter("b1", [C], f32, isOutput=False)
    b2_in = nc.declare_dram_parameter("b2", [C], f32, isOutput=False)
    ilo_in = nc.declare_dram_parameter("ilo", [S, 128, cap_lo // 16], i16, isOutput=False)
    ihi_in = nc.declare_dram_parameter("ihi", [S, 128, cap_hi // 16], i16, isOutput=False)
    ia_in = nc.declare_dram_parameter("ia", [S, 128, cap // 16], i16, isOutput=False)
    slot_in = nc.declare_dram_parameter("slot", [S, 128, nb], f32, isOutput=False)
    out_ext = nc.declare_dram_parameter("out", [NLOC, C], f32, isOutput=True)

    # ---- internal DRAM
    table = nc.dram_tensor("table_d", [N, TROW], bf16)
    adst_tbl = nc.dram_tensor("adst_d", [NLOC + 1, AROW], bf16)
    ar_in = nc.dram_tensor("ar_in_d", [1, H], f32)
    ar_out = nc.dram_tensor("ar_out_d", [1, H], f32, addr_space="Shared")
    ag_in = nc.dram_tensor("ag_in_d", [C, NLOC], bf16)
    actT_full = nc.dram_tensor("actT_full_d", [N_CORES * C, NLOC], bf16, addr_space="Shared")

    G = 8  # tiles per build slab

    with TileContext(nc) as tc:
        with (
            tc.tile_pool(name="const", bufs=1) as cpool,
            tc.tile_pool(name="bld", bufs=3) as bld,
            tc.tile_pool(name="bldp", bufs=2, space="PSUM") as bldp,
            tc.tile_pool(name="gat", bufs=3) as gat,
            tc.tile_pool(name="edge", bufs=2) as edge,
            tc.tile_pool(name="up", bufs=2, space="PSUM") as upool,
            tc.tile_pool(name="fix", bufs=1) as fix,
        ):
            nc.gpsimd.load_library(library_config.mlp)

            # ============ constants ============
            ident = cpool.tile([128, 128], bf16)
            make_identity(nc, ident[:])
            iota_i = cpool.tile([128, 128], i32)
            nc.gpsimd.iota(iota_i[:], pattern=[[1, 128]], base=0, channel_multiplier=0)
            iota3 = cpool.tile([128, 1, 128], bf16)
            nc.vector.tensor_copy(out=iota3[:, 0, :], in_=iota_i[:])
            ones_row = cpool.tile([1, 128], f32)
            nc.vector.memset(ones_row[:], 1.0)
            ones_col = cpool.tile([128, 1], f32)
            nc.vector.memset(ones_col[:], 1.0)

            # all idx/slot arrays, resident (layer-independent)
            ilo_all = cpool.tile([128, S, cap_lo // 16], i16)
            ihi_all = cpool.tile([128, S, cap_hi // 16], i16)
            ia_all = cpool.tile([128, S, cap // 16], i16)
            nc.sync.dma_start(out=ilo_all[:], in_=ilo_in.ap().rearrange("s p w -> p s w"))
            nc.sync.dma_start(out=ihi_all[:], in_=ihi_in.ap().rearrange("s p w -> p s w"))
            nc.sync.dma_start(out=ia_all[:], in_=ia_in.ap().rearrange("s p w -> p s w"))
            slot4 = cpool.tile([128, S, nb, 1], f32)
            nc.sync.dma_start(
                out=slot4[:, :, :, 0], in_=slot_in.ap().rearrange("s p w -> p s w")
            )

            # own-node x^T, resident (adst layer-1 lhsT)
            xto = cpool.tile([128, NLOC], bf16)
            nc.sync.dma_start(out=xto[:], in_=xTo_in[:, :])

            # bias broadcast tiles [128, 1, C]
            b1b = cpool.tile([128, 1, C], f32)
            nc.sync.dma_start(out=b1b[:, 0, :], in_=b1_in.ap().partition_broadcast(128))
            b2b = cpool.tile([128, 1, C], f32)
            nc.sync.dma_start(out=b2b[:, 0, :], in_=b2_in.ap().partition_broadcast(128))

            # sentinel row of adst table
            sent_t = cpool.tile([1, AROW], bf16)
            nc.vector.memset(sent_t[:], -1.0e9)
            nc.sync.dma_start(out=adst_tbl[SENT_A : SENT_A + 1, :], in_=sent_t[:])

            # ---- R matrices: R = [W^T | Vsrc' | Vdst']  (bf16, [K=kdim, 132])
            def build_R(w_dram, att_dram, kdim):
                wt = bld.tile([128, kdim], f32, tag="wld")
                nc.sync.dma_start(out=wt[:, :], in_=w_dram[:])          # [HC, kdim]
                wb = bld.tile([128, kdim], bf16, tag="wldb")
                nc.vector.tensor_copy(out=wb[:], in_=wt[:])
                vsd = cpool.tile([128, 4], f32, tag="vsd")  # cols 0:2 Vsrc, 2:4 Vdst
                nc.vector.memset(vsd[:], 0.0)
                for h in range(H):
                    nc.sync.dma_start(
                        out=vsd[h * C : (h + 1) * C, h : h + 1],
                        in_=att_dram[0:1, h, C : 2 * C].rearrange("o c -> c o"),
                    )
                    nc.sync.dma_start(
                        out=vsd[h * C : (h + 1) * C, 2 + h : 3 + h],
                        in_=att_dram[0:1, h, 0:C].rearrange("o c -> c o"),
                    )
                vsdb = cpool.tile([128, 4], bf16, tag="vsdb")
                nc.vector.tensor_copy(out=vsdb[:], in_=vsd[:])
                r_ps = bldp.tile([128, 132], f32, tag="ps", space="PSUM")
                nc.tensor.transpose(
                    out=r_ps[:kdim, 0:128].bitcast(bf16)[:, 0:128], in_=wb[:, :],
                    identity=ident[:],
                )
                nc.tensor.matmul(out=r_ps[:kdim, 128:132], lhsT=wb[:, :], rhs=vsdb[:, :],
                                 start=True, stop=True)
                r_sb = cpool.tile([128, 132], bf16, tag=f"R{kdim}")
                nc.vector.tensor_copy(out=r_sb[:kdim, 0:128],
                                      in_=r_ps[:kdim, 0:128].bitcast(bf16)[:, 0:128])
                nc.vector.tensor_copy(out=r_sb[:kdim, 128:132], in_=r_ps[:kdim, 128:132])
                return r_sb

            R1 = build_R(w1_in, att1_in, FIN)
            R2 = build_R(w2_in, att2_in, C)

            # resident state
            U_sb = cpool.tile([128, S, HC], f32)        # aggregation output per layer
            actT_sb = cpool.tile([C, S, 128], bf16)     # layer-1 activations^T (own)
            gacc = cpool.tile([128, H], f32)

            # ============ table build (replicated; all N rows) ============
            def build_table(layer):
                kdim = FIN if layer == 1 else C
                R = R1 if layer == 1 else R2
                # slab list: (src_ap_fn, row0, width)
                if layer == 1:
                    blocks = [(None, 0, N)]
                else:
                    blocks = [(k, k * NLOC, NLOC) for k in range(N_CORES)]
                for kblk, row0, width in blocks:
                    off = 0
                    while off < width:
                        w = min(G * 128, width - off)
                        ntiles_full, rem = divmod(w, 128)
                        xs = bld.tile([128, G * 128], bf16, tag="xs")
                        if layer == 1:
                            nc.sync.dma_start(out=xs[:kdim, :w],
                                              in_=xT_in[:, off : off + w])
                        else:
                            nc.sync.dma_start(
                                out=xs[:kdim, :w],
                                in_=actT_full[kblk * C : (kblk + 1) * C,
                                              off : off + w])
                        tr = bld.tile([128, G, 132], bf16, tag="tr")
                        nt = ntiles_full + (1 if rem else 0)
                        for u in range(nt):
                            rows = 128 if u < ntiles_full else rem
                            hp = bldp.tile([128, 132], f32, tag="hp", space="PSUM")
                            nc.tensor.matmul(
                                out=hp[:rows, :],
                                lhsT=xs[:kdim, u * 128 : u * 128 + rows],
                                rhs=R[:kdim, :], start=True, stop=True)
                            nc.vector.tensor_copy(out=tr[:rows, u, 0:132],
                                                  in_=hp[:rows, :])
                        r0 = row0 + off
                        if ntiles_full:
                            nc.scalar.dma_start(
                                out=table[r0 : r0 + ntiles_full * 128, 0:130]
                                    .rearrange("(u p) f -> p u f", p=128),
                                in_=tr[:, 0:ntiles_full, 0:130])
                        if rem:
                            nc.scalar.dma_start(
                                out=table[r0 + ntiles_full * 128 :
                                          r0 + ntiles_full * 128 + rem, 0:130],
                                in_=tr[:rem, ntiles_full, 0:130])
                        off += w

            # ============ adst build (own nodes) ============
            def build_adst(layer):
                kdim = FIN if layer == 1 else C
                R = R1 if layer == 1 else R2
                GA = 4
                for g0 in range(0, S, GA):
                    gn = min(GA, S - g0)
                    ap_ = bldp.tile([128, GA, 2], f32, tag="ps", space="PSUM")
                    for u in range(gn):
                        t = g0 + u
                        if layer == 1:
                            lhsT = xto[:, t * NST_NODES : (t + 1) * NST_NODES]
                        else:
                            lhsT = actT_sb[:, t, 0:NST_NODES]
                        nc.tensor.matmul(out=ap_[:NST_NODES, u, :], lhsT=lhsT,
                                         rhs=R[:kdim, 130:132], start=True, stop=True)
                    asb = bld.tile([128, GA, 2], bf16, tag="asb")
                    nc.vector.tensor_copy(out=asb[:NST_NODES, 0:gn, :],
                                          in_=ap_[:NST_NODES, 0:gn, :])
                    nc.scalar.dma_start(
                        out=adst_tbl[g0 * NST_NODES : (g0 + gn) * NST_NODES, 0:2]
                            .rearrange("(u p) f -> p u f", p=NST_NODES),
                        in_=asb[:NST_NODES, 0:gn, :])

            # ============ edge pass ============
            qctr = [0]

            def _gq():
                # Tile assigns SWDGE sem lanes per Pool-DMA call in SCHEDULED
                # order, which the scheduler may permute vs emission order - no
                # emission-time rotation can stay consistent. One queue always
                # is, and a single SWDGE queue already stripes descriptors
                # across all 16 DMA engines.
                return 0

            GMAX = int(__import__("os").environ.get("KGMAX", "8"))

            def chunked_gather(dst, blk0, nblk, src, idx_all, t, row):
                c = 0
                while c < nblk:
                    n = min(GMAX, nblk - c)
                    nc.gpsimd.dma_gather(
                        dst[:, blk0 + c : blk0 + c + n, :], src,
                        idx_all[:, t, (c * 8) : (c + n) * 8],
                        n * 128, n * 128, row, queue_num=_gq())
                    c += n

            def edge_pass(layer):
                nc.vector.memset(gacc[:], 0.0)
                for t in range(S):
                    g_t = gat.tile([128, nb, TROW], bf16, tag="G")
                    chunked_gather(g_t, 0, nb_lo, table[0:V_HALF, :], ilo_all, t, TROW)
                    chunked_gather(g_t, nb_lo, nb_hi, table[V_HALF:N, :], ihi_all, t, TROW)
                    a_t = gat.tile([128, nb, AROW], bf16, tag="A")
                    chunked_gather(a_t, 0, nb, adst_tbl[:], ia_all, t, AROW)
                    # scores: s = asrc + adst -> lrelu -> exp (+gsum accum)
                    s_t = edge.tile([128, nb, 2], f32, tag="s")
                    nc.vector.tensor_tensor(out=s_t[:], in0=g_t[:, :, HC : HC + 2],
                                            in1=a_t[:, :, 0:2], op=Alu.add)
                    neg_t = edge.tile([128, nb, 2], f32, tag="ng")
                    nc.vector.tensor_scalar(out=neg_t[:], in0=s_t[:], scalar1=0.0,
                                            scalar2=0.2, op0=Alu.min, op1=Alu.mult)
                    sl_t = edge.tile([128, nb, 2], f32, tag="sl")
                    nc.vector.scalar_tensor_tensor(out=sl_t[:], in0=s_t[:], scalar=0.0,
                                                   in1=neg_t[:], op0=Alu.max,
                                                   op1=Alu.add)
                    w_b = edge.tile([128, nb, 2, 1], bf16, tag="wb")
                    acc_t = edge.tile([128, H], f32, tag="acc")
                    for h in range(H):
                        nc.scalar.activation(
                            out=w_b[:, :, h, :], in_=sl_t[:, :, h : h + 1],
                            func=Act.Exp, accum_out=acc_t[:, h : h + 1])
                    nc.vector.tensor_tensor(out=gacc[:], in0=gacc[:], in1=acc_t[:],
                                            op=Alu.add)
                    # messages = h * w  (one op, w broadcast per head)
                    msg = edge.tile([128, nb, H, C], bf16, tag="m")
                    nc.vector.tensor_tensor(
                        out=msg[:],
                        in0=g_t[:, :, 0:HC].rearrange("p j (h c) -> p j h c", h=H),
                        in1=w_b[:].to_broadcast([128, nb, H, C]), op=Alu.mult)
                    # one-hot for all nb blocks in one op
                    oh = edge.tile([128, nb, 128], bf16, tag="oh")
                    nc.vector.tensor_tensor(
                        out=oh[:], in0=iota3[:].to_broadcast([128, nb, 128]),
                        in1=slot4[:, t, :, :].to_broadcast([128, nb, 128]),
                        op=Alu.is_equal)
                    u_ps = upool.tile([128, HC], f32, tag="U", space="PSUM")
                    for j in range(nb):
                        nc.tensor.matmul(
                            out=u_ps[:], lhsT=oh[:, j, :], rhs=msg[:, j, :, :],
                            start=(j == 0), stop=(j == nb - 1))
                    nc.scalar.activation(out=U_sb[:, t, :], in_=u_ps[:], func=Act.Copy)

            # ============ stats allreduce + fixup ============
            def stats_and_fixup(layer):
                g_ps = bldp.tile([128, H], f32, tag="ps", space="PSUM")
                nc.tensor.matmul(out=g_ps[0:1, :], lhsT=ones_col[:], rhs=gacc[:],
                                 start=True, stop=True)
                g_sb = fix.tile([1, H], f32, tag="gsb")
                nc.vector.tensor_copy(out=g_sb[:], in_=g_ps[0:1, :])
                nc.sync.dma_start(out=ar_in[:], in_=g_sb[:])
                tc.strict_bb_all_engine_barrier()
                nc.gpsimd.collective_compute(
                    "AllReduce", mybir.AluOpType.add,
                    replica_groups=[list(range(N_CORES))],
                    ins=[ar_in[:]], outs=[ar_out[:]],
                )
                tg = fix.tile([1, H], f32, tag="tg")
                nc.sync.dma_start(out=tg[:], in_=ar_out[:])
                tb_ps = bldp.tile([128, H], f32, tag="ps", space="PSUM")
                nc.tensor.matmul(out=tb_ps[:], lhsT=ones_row[0:1, :], rhs=tg[:],
                                 start=True, stop=True)
                tb = fix.tile([128, H], f32, tag="tb")
                nc.vector.tensor_scalar(out=tb[:], in0=tb_ps[:], scalar1=1.0e-10,
                                        scalar2=None, op0=Alu.add)
                rt = fix.tile([128, H], f32, tag="rt")
                nc.vector.reciprocal(out=rt[:], in_=tb[:])
                nc.vector.tensor_scalar(out=rt[:], in0=rt[:], scalar1=0.5,
                                        scalar2=None, op0=Alu.mult)

                bias = b1b if layer == 1 else b2b
                # m = U0*rt0 + U1*rt1 + bias   (batched over all supertiles)
                m1 = fix.tile([128, S, C], f32, tag="m1")
                nc.vector.tensor_scalar(out=m1[:], in0=U_sb[:, :, C:HC],
                                        scalar1=rt[:, 1:2], scalar2=None, op0=Alu.mult)
                m0 = fix.tile([128, S, C], f32, tag="m0")
                nc.vector.scalar_tensor_tensor(out=m0[:], in0=U_sb[:, :, 0:C],
                                               scalar=rt[:, 0:1], in1=m1[:],
                                               op0=Alu.mult, op1=Alu.add)
                nc.vector.tensor_tensor(out=m0[:], in0=m0[:],
                                        in1=bias[:].to_broadcast([128, S, C]),
                                        op=Alu.add)
                if layer == 1:
                    act_n = fix.tile([128, S, C], bf16, tag="an")
                    nc.scalar.activation(out=act_n[:], in_=m0[:], func=Act.Relu)
                    for t in range(S):
                        tp = bldp.tile([64, 128], f32, tag="ps", space="PSUM")
                        nc.tensor.transpose(
                            out=tp[:, 0:128].bitcast(bf16)[:, 0:NST_NODES],
                            in_=act_n[0:NST_NODES, t, :],
                            identity=ident[:NST_NODES, :NST_NODES])
                        nc.scalar.activation(
                            out=actT_sb[:, t, 0:NST_NODES],
                            in_=tp[:, 0:128].bitcast(bf16)[:, 0:NST_NODES],
                            func=Act.Copy)
                    nc.sync.dma_start(
                        out=ag_in[:, :].rearrange("c (t p) -> c t p", t=S),
                        in_=actT_sb[:, :, 0:NST_NODES])
                else:
                    nc.sync.dma_start(
                        out=out_ext[:, :].rearrange("(t p) c -> p t c", p=NST_NODES),
                        in_=m0[0:NST_NODES, :, :])

            # ============ main sequence ============
            if phases >= 1:
                build_table(1)
                build_adst(1)
            if phases >= 2:
                tc.strict_bb_all_engine_barrier()
                edge_pass(1)
            if phases >= 3:
                stats_and_fixup(1)
            if phases >= 4:
                tc.strict_bb_all_engine_barrier()
                nc.gpsimd.collective_compute(
                    "AllGather", mybir.AluOpType.bypass,
                    replica_groups=[list(range(N_CORES))],
                    ins=[ag_in[:]], outs=[actT_full[:]],
                )
            if phases >= 5:
                tc.strict_bb_all_engine_barrier()
                build_table(2)
                build_adst(2)
            if phases >= 6:
                tc.strict_bb_all_engine_barrier()
                edge_pass(2)
            if phases >= 7:
                stats_and_fixup(2)

    # Re-assign SWDGE queues in SCHEDULED order: Tile gives Pool-DMA call i the
    # DMASW sem lane i%8 (scheduled order), and a lane's sem may only ever fire
    # from one queue -> queue must be i%4 in the same order. Emission-time
    # assignment can't know the schedule, so rewrite after scheduling.
    idx = 0
    for bb in nc.m.functions[0].blocks:
        for inst in bb.instructions:
            if isinstance(inst, mybir.InstDMAGatherAnt):
                inst.queue_num = idx % 4
                idx += 1

    nc.compile()
    return nc


# --------------------------------------------------------------------------
# entry point
# --------------------------------------------------------------------------

def _make_in_maps(x, edge_index, W1, att1, b1, W2, att2, b2):
    import ml_dtypes

    x = np.asarray(x, np.float32)
    xT = np.ascontiguousarray(x.T).astype(ml_dtypes.bfloat16)
    ilo, ihi, ia, slot, nb_lo, nb_hi = _preprocess(np.asarray(edge_index))
    common = {
        "xT": xT,
        "W1": np.asarray(W1, np.float32), "W2": np.asarray(W2, np.float32),
        "att1": np.asarray(att1, np.float32), "att2": np.asarray(att2, np.float32),
        "b1": np.asarray(b1, np.float32), "b2": np.asarray(b2, np.float32),
    }
    in_maps = []
    for k in range(N_CORES):
        m = dict(common)
        m["xTo"] = np.ascontiguousarray(xT[:, k * NLOC : (k + 1) * NLOC])
        m["ilo"], m["ihi"], m["ia"], m["slot"] = ilo[k], ihi[k], ia[k], slot[k]
        in_maps.append(m)
    return in_maps, (nb_lo, nb_hi)


def kernel(x, edge_index, W1, att1, b1, W2, att2, b2):
    global _compiled
    from concourse.bass_utils import run_bass_kernel_spmd

    in_maps, key = _make_in_maps(x, edge_index, W1, att1, b1, W2, att2, b2)
    if _compiled is None or _compiled[1] != key:
        nc = _build_program(*key)
        _compiled = (nc, key)
    nc = _compiled[0]

    res = run_bass_kernel_spmd(nc, in_maps, list(range(N_CORES)))
    out = np.concatenate([res.results[k]["out"] for k in range(N_CORES)], axis=0)
    return out
